# revision 23
# baseline (speedup 1.0000x reference)
"""GPTQ-style grouped-dequant linear on 8 Trainium2 cores.

out[m,n] = sum_k A[m,k] * (q[n,k] - zeros[n,k//128]) * scales[n,k//128] + bias[n]
M=2048, K=4096, N=4096, group=128.

Sharding: column-parallel — qweight/scales/zeros/bias split along N (512/core),
A replicated. Host does layout permutes + dtype casts only: A pre-cast to bf16
(same rounding the device matmul path applies anyway), q repacked to uint8,
scales/zeros pre-broadcast along the 128 k-partitions (pure replication) so
the device spends no PE time on rank-1 broadcasts.

Per core: dequant is two DVE tensor_tensor ops per k-group producing bf16 W^T
tiles in [k,n] layout; scales/zeros stream through a small rotating SBUF
window. The only PE work is the 512 productive 128x128x512 matmuls (16
m-tiles x 32 k-groups), kept dense by 8 staggered lead tiles with catch-up
bursts; a short dummy-MM spin releases the HAM clock gate while the first
DMAs land. DMA: everything warmup-critical rides the sync queue in exact
consumption order (delivery order == issue order on one queue); q chunks ride
gpsimd; late phase-2 A tiles ride scalar, gated by lead-buffer release so
they cannot steal warmup bandwidth; outputs ride sync (idle by then). Bias is
folded into the PSUM->SBUF eviction (DVE add); output is bf16, upcast on
host.
"""

import numpy as np
import ml_dtypes

import concourse.bass as bass
import concourse.mybir as mybir
import concourse.tile as tile
from concourse import bacc
from concourse.bass_utils import run_bass_kernel_spmd

P = 128
M, K, N = 2048, 4096, 4096
NCORES = 8
NS = N // NCORES          # 512 out-features per core
G = K // P                # 32 groups (group_size == P == 128)
MT = M // P               # 16 output row tiles

NLEAD = 8                 # lead m-tiles resident in PSUM during warmup
JOIN_AT = {0: 0, 1: 4, 2: 7, 3: 10, 4: 14, 5: 18, 6: 22, 7: 27}
SZCHUNK = 4               # groups per scales/zeros chunk (rotating window)
NDUMMY = 10               # warmup matmuls to release the HAM clock gate
NPRE = 2                  # phase-2 A tiles prefetched on the sync queue

_cached = None


def _build():
    nc = bacc.Bacc("TRN2", target_bir_lowering=False, debug=False,
                   num_devices=NCORES)
    at = nc.dram_tensor("AT4", [MT, P, G, P], mybir.dt.bfloat16,
                        kind="ExternalInput")
    qt = nc.dram_tensor("q4", [P, G, NS], mybir.dt.uint8,
                        kind="ExternalInput")
    st = nc.dram_tensor("srep", [P, G, NS], mybir.dt.bfloat16,
                        kind="ExternalInput")
    zt = nc.dram_tensor("zrep", [P, G, NS], mybir.dt.bfloat16,
                        kind="ExternalInput")
    bi = nc.dram_tensor("brep", [P, NS], mybir.dt.float32,
                        kind="ExternalInput")
    out = nc.dram_tensor("out", [M, NS], mybir.dt.bfloat16,
                         kind="ExternalOutput")

    bf16, f32 = mybir.dt.bfloat16, mybir.dt.float32
    NCH = G // SZCHUNK

    with tile.TileContext(nc) as tc:
        with (
            tc.tile_pool(name="const", bufs=1) as const,
            tc.tile_pool(name="qpool", bufs=1) as qpool,
            tc.tile_pool(name="szpool", bufs=3) as szpool,
            tc.tile_pool(name="tmp", bufs=3) as tmpp,
            tc.tile_pool(name="wt", bufs=1) as wtp,
            tc.tile_pool(name="apool", bufs=NLEAD + NPRE) as apool,
            tc.tile_pool(name="mpsum", bufs=8, space="PSUM") as mpsum,
            tc.tile_pool(name="opool", bufs=3) as opool,
        ):
            q8s = qpool.tile([P, G, NS], mybir.dt.uint8, tag="q8s")
            bias_r = const.tile([P, NS], f32, tag="bias_r")
            scratch = const.tile([P, NS], bf16, tag="scratch")
            nc.gpsimd.memset(scratch, 0.0)
            sr, zr, qr = st.ap(), zt.ap(), qt.ap()
            atr = at.ap()  # [MT, P, G, P], per-partition contiguous

            def load_ab(mt, eng, split=1):
                ab = apool.tile([P, G, P], bf16)
                for h in range(split):
                    g0, g1 = h * (G // split), (h + 1) * (G // split)
                    eng.dma_start(out=ab[:, g0:g1, :], in_=atr[mt, :, g0:g1, :])
                return ab

            def load_sz(c):
                g0, g1 = c * SZCHUNK, (c + 1) * SZCHUNK
                sc = szpool.tile([P, SZCHUNK, NS], bf16)
                nc.sync.dma_start(out=sc[:], in_=sr[:, g0:g1, :])
                zc = szpool.tile([P, SZCHUNK, NS], bf16)
                nc.sync.dma_start(out=zc[:], in_=zr[:, g0:g1, :])
                return sc, zc

            # sync queue, delivery order == consumption order (the rotating
            # szpool window self-gates later chunks, so issuing all of them
            # up front steals no bandwidth); q rides the same stream so
            # nothing competes for HBM out of order
            def load_q(g0, g1):
                nc.sync.dma_start(out=q8s[:, g0:g1, :], in_=qr[:, g0:g1, :])

            load_q(0, 4)
            szt = [None] * NCH
            szt[0] = load_sz(0)
            lead_ab = [None] * NLEAD
            lead_ab[0] = load_ab(0, nc.sync, split=2)
            lead_ab[1] = load_ab(1, nc.sync)
            for c in range(1, NCH):
                if c == 1:
                    load_q(4, 8)
                elif c == 2:
                    load_q(8, 16)
                elif c == 4:
                    load_q(16, 32)
                szt[c] = load_sz(c)
                if c + 1 < NLEAD:
                    lead_ab[c + 1] = load_ab(c + 1, nc.sync)
            nc.sync.dma_start(out=bias_r[:], in_=bi.ap()[:])
            pre = [load_ab(mt, nc.sync) for mt in range(NLEAD, NLEAD + NPRE)]

            def new_ps():
                ps = mpsum.tile([P, NS], f32)
                return ps

            # warmup spin: release the HAM clock gate while DMA streams in
            dummy_ps = new_ps()
            for i in range(NDUMMY):
                nc.tensor.matmul(dummy_ps[:], scratch[:, :P], scratch[:],
                                 start=(i == 0), stop=(i == NDUMMY - 1))

            lead_ps = [new_ps() for _ in range(NLEAD)]

            def finish(mt, ps):
                ob = opool.tile([P, NS], bf16)
                nc.vector.tensor_tensor(ob[:], ps[:], bias_r[:],
                                        mybir.AluOpType.add)
                nc.gpsimd.dma_start(out=out.ap()[mt * P:(mt + 1) * P, :],
                                    in_=ob[:])

            # Phase 1: dequant each k-group on DVE, immediately consumed by
            # the lead tiles' PSUM accumulation chains (catch-up bursts as
            # each lead joins keep the PE dense).
            wts = []
            for g in range(G):
                sc, zc = szt[g // SZCHUNK]
                j = g % SZCHUNK
                tmp = tmpp.tile([P, NS], bf16)
                nc.vector.tensor_tensor(tmp[:], q8s[:, g, :], zc[:, j, :],
                                        mybir.AluOpType.subtract)
                wt = wtp.tile([P, NS], bf16, tag=f"wt{g}")
                nc.vector.tensor_tensor(wt[:], tmp[:], sc[:, j, :],
                                        mybir.AluOpType.mult)
                wts.append(wt)
                for mt in range(NLEAD):
                    if JOIN_AT[mt] == g:
                        for gc in range(g + 1):  # catch-up burst
                            nc.tensor.matmul(lead_ps[mt][:],
                                             lead_ab[mt][:, gc, :], wts[gc][:],
                                             start=(gc == 0),
                                             stop=(gc == G - 1))
                    elif JOIN_AT[mt] < g:
                        nc.tensor.matmul(lead_ps[mt][:], lead_ab[mt][:, g, :],
                                         wt[:], start=False,
                                         stop=(g == G - 1))

            # Late phase-2 A tiles on the scalar queue: with the apool sized
            # NLEAD + NPRE, each is gated on a lead tile's last read, so none
            # of them steal warmup bandwidth.
            pre += [load_ab(mt, nc.scalar) for mt in range(NLEAD + NPRE, MT)]

            for mt in range(NLEAD):
                finish(mt, lead_ps[mt])

            # Phase 2: remaining output tiles, dense back-to-back matmuls
            for mt in range(NLEAD, MT):
                ab = pre[mt - NLEAD]
                ps = new_ps()
                for g in range(G):
                    nc.tensor.matmul(ps[:], ab[:, g, :], wts[g][:],
                                     start=(g == 0), stop=(g == G - 1))
                finish(mt, ps)

    nc.compile()
    return nc


def _prep_inputs(A, qweight, scales, zeros, bias):
    # AT4[mt, p, g, j] = A[mt*128+j, g*128+p]  (layout permute + bf16 cast)
    at4 = np.ascontiguousarray(
        A.reshape(MT, P, G, P).transpose(0, 3, 2, 1).astype(ml_dtypes.bfloat16))
    in_maps = []
    for c in range(NCORES):
        r = slice(c * NS, (c + 1) * NS)
        # q4[p, g, n] = q[n, g*128+p]
        q4 = np.ascontiguousarray(
            qweight[r].astype(np.uint8).T.reshape(G, P, NS).transpose(1, 0, 2))
        # scales/zeros pre-broadcast across the 128 k-partitions (replication)
        srep = np.ascontiguousarray(np.broadcast_to(
            scales[r].T.astype(ml_dtypes.bfloat16)[None, :, :], (P, G, NS)))
        zrep = np.ascontiguousarray(np.broadcast_to(
            zeros[r].T.astype(ml_dtypes.bfloat16)[None, :, :], (P, G, NS)))
        brep = np.ascontiguousarray(np.broadcast_to(
            bias[r].astype(np.float32)[None, :], (P, NS)))
        in_maps.append({
            "AT4": at4,
            "q4": q4,
            "srep": srep,
            "zrep": zrep,
            "brep": brep,
        })
    return in_maps


def run(inputs, **spmd_kwargs):
    global _cached
    if _cached is None:
        _cached = _build()
    in_maps = _prep_inputs(**inputs)
    res = run_bass_kernel_spmd(_cached, in_maps, list(range(NCORES)),
                               **spmd_kwargs)
    outp = np.concatenate(
        [res.results[c]["out"].astype(np.float32) for c in range(NCORES)],
        axis=1)
    return outp, res


def kernel(**inputs):
    return run(inputs)[0]
